# revision 28
# baseline (speedup 1.0000x reference)
"""Column-wise RMS normalization on 8 Trainium2 NeuronCores.

Computes y = x * rsqrt(sum(x*x, axis=0) + eps) for x [32768, 2048] f32.

Sharding: column-parallel — each core owns 256 columns (no collectives).
The host casts to fp16 (rel-err budget 2e-2 dwarfs fp16 rounding) and
re-packs the core's shard as two stacked column-half planes
[2N, 128] = [rows of cols 0:128; rows of cols 128:256], so every DMA
group [128p, g, 128c] moves 4KB contiguous runs per partition. HBM
traffic is 16MB in + 16MB out per core, the bandwidth floor.

Two-stage column-half pipeline: stage a (cols 0:128) streams in and
reduces; while stage b streams in, stage a's scaled output streams OUT
simultaneously on a different HWDGE queue — input and output DMA
overlap, hiding the square/reduce drain and the scale latency that a
single-stage schedule exposes as a dead-DMA gap.

Per stage:
  pass A: squares into fp8(e4m3) scratch (ACT takes 10/15 big groups at
          ~141G elem/s, DVE the rest at ~118G); column reduction on
          TensorE via fp8 DoubleRow ones-matmuls, 8 rows per matmul,
          accumulating 4 interleaved 128-col partials in PSUM [16, 512].
  scale:  4-partial reduce + Sqrt(+eps) + reciprocal on [1, 128], K=1
          ones-matmul broadcast to PSUM, materialized as [128, 16, 128]
          fp16 so pass-B muls are equal-shape stride-1 fp16 ops.
  pass B: y = x*s on DVE tensor_tensor (~230G elem/s), fp16 out tiles.

Queues: all in-DMAs + stage-b out-DMAs on SP; stage-a out-DMAs on the
ACT HWDGE queue (they run concurrently with stage-b in-DMAs).
"""

import numpy as np

import concourse.bacc as bacc
import concourse.bass as bass
import concourse.tile as tile
from concourse import mybir
from concourse.bass_utils import run_bass_kernel_spmd

N, D = 32768, 2048
EPS = 1e-6
NCORES = 8
C = D // NCORES  # 256 columns per core
H = 128          # column-half width
P = 128          # partitions
T = N // P       # 256 rows per partition
G = 32           # row-group (t) per DMA / compute chunk

# Stage a ramps up small (fast square->matmul pipeline spin-up, PE is
# the scale gate) and ends small (short final chain); stage b starts
# warm and ends small. ACT takes ~half the big groups' squares.
IN_GROUPS_A = [
    (0, 4), (4, 8), (12, 16), (28, 32), (60, 32), (92, 32), (124, 32),
    (156, 32), (188, 32), (220, 16), (236, 8), (244, 8), (252, 2), (254, 2),
]
ACT_GROUPS_A = {2, 4, 6, 7, 8}
IN_GROUPS_B = [(j * G, G) for j in range(7)] + [
    (224, 16), (240, 8), (248, 4), (252, 2), (254, 2),
]
ACT_GROUPS_B = {0, 1, 2, 3, 4, 5, 6, 7, 8, 9, 10, 11}
OUT_GROUPS = [(0, 8), (8, 8), (16, 16)] + [(32 + j * G, G) for j in range(6)] + [(224, 16), (240, 8), (248, 4), (252, 4)]

_NC = None


def _build() -> bass.Bass:
    nc = bacc.Bacc("TRN2", target_bir_lowering=False, enable_partition_id=False)
    x = nc.dram_tensor("x", [2 * N, H], mybir.dt.float16, kind="ExternalInput")
    y = nc.dram_tensor("y", [2 * N, H], mybir.dt.float16, kind="ExternalOutput")
    xv = x[:, :].rearrange("(h p t) c -> h p t c", h=2, p=P)
    yv = y[:, :].rearrange("(h p t) c -> h p t c", h=2, p=P)

    with tile.TileContext(nc) as tc:
        with (
            tc.tile_pool(name="cache", bufs=1) as cachep,
            tc.tile_pool(name="consts", bufs=1) as consts,
            tc.tile_pool(name="sq", bufs=2) as sqp,
            tc.tile_pool(name="outs", bufs=4) as outp,
            tc.tile_pool(name="scale", bufs=1) as scalep,
            tc.tile_pool(name="ps", bufs=1, space="PSUM") as psp,
        ):
            xc = cachep.tile([P, 2, T, H], mybir.dt.float16)
            ones2 = consts.tile([P, 2, P], mybir.dt.float8e4)
            nc.vector.memset(ones2, 1.0)
            eps_t = consts.tile([P, 1], mybir.dt.float32)
            nc.vector.memset(eps_t, EPS)
            # Pre-warm ACT function tables off the critical path.
            warm = consts.tile([1, 1], mybir.dt.float32)
            nc.scalar.activation(
                out=warm, in_=eps_t[0:1, 0:1],
                func=mybir.ActivationFunctionType.Square,
            )
            nc.scalar.activation(
                out=warm, in_=eps_t[0:1, 0:1],
                func=mybir.ActivationFunctionType.Sqrt,
            )

            u_ps = [psp.tile([P, 4 * H], mybir.dt.float32, name=f"u_ps{h}") for h in range(2)]
            smax = [scalep.tile([P, 1, H], mybir.dt.float16, name=f"smax{h}") for h in range(2)]

            nmms = []  # matmul count per half (filled on first pass)

            def in_group(h, gi, t0, g, act_groups):
                ts_ = slice(t0, t0 + g)
                nc.sync.dma_start(out=xc[:, h, ts_, :], in_=xv[h][:, ts_, :])
                if g > 2:
                    sq = sqp.tile([P, g, H], mybir.dt.float8e4, tag="sq", bufs=5)
                else:
                    sq = sqp.tile([P, g, H], mybir.dt.float8e4, tag="sqt", bufs=4)
                if gi in act_groups:
                    nc.scalar.activation(
                        out=sq,
                        in_=xc[:, h, ts_, :],
                        func=mybir.ActivationFunctionType.Square,
                    )
                else:
                    nc.vector.tensor_mul(sq, xc[:, h, ts_, :], xc[:, h, ts_, :])
                # 8-row DoubleRow matmuls: rhs viewed [P, 2, (r/2)*H] (the
                # required Num=2 AP) pairs row t with t+r/2; accumulates
                # r/2 interleaved 128-col partials into u_ps[:, :inner].
                mm = []
                for r0 in range(0, g, 8):
                    r = min(8, g - r0)
                    view = sq[:, r0 : r0 + r, :].rearrange(
                        "p (two t) c -> p two (t c)", two=2
                    )
                    mm.append((view, (r // 2) * H))
                return mm

            def scale_chain(h):
                # u is redundantly materialized on every partition (M=128
                # ones), so the scale is all-local: fold 4 partials, sqrt,
                # reciprocal -- no PE broadcast hop.
                u2 = scalep.tile([P, 1, H], mybir.dt.float32, name=f"u{h}")
                uview = u_ps[h][:, :].rearrange("p (q c) -> p c q", q=4)
                nc.vector.reduce_sum(u2[:, 0, :], uview, axis=mybir.AxisListType.X)
                tsq = scalep.tile([P, 1, H], mybir.dt.float32, name=f"t{h}")
                nc.scalar.activation(
                    out=tsq[:, 0, :],
                    in_=u2[:, 0, :],
                    func=mybir.ActivationFunctionType.Sqrt,
                    bias=eps_t[:, 0:1],
                    scale=1.0,
                )
                s32 = scalep.tile([P, 1, H], mybir.dt.float32, name=f"s32_{h}")
                nc.vector.reciprocal_approx_fast(out=s32[:, :, :], in_=tsq[:, :, :])
                nc.vector.tensor_copy(smax[h], s32[:, :, :])

            def emit_mms(h, mm, kstate):
                for rhs, inner in mm:
                    nc.tensor.matmul(
                        u_ps[h][:, :inner],
                        lhsT=ones2[:, :, :],
                        rhs=rhs,
                        start=(kstate[0] == 0),
                        stop=(kstate[0] == kstate[1] - 1),
                        perf_mode=mybir.MatmulPerfMode.DoubleRow,
                    )
                    kstate[0] += 1

            def out_group(h, t0, g, engine, mul_engine=None):
                ts_ = slice(t0, t0 + g)
                ot = outp.tile([P, g, H], mybir.dt.float16, tag="ot", bufs=6)
                (mul_engine or nc.vector).tensor_mul(
                    ot, xc[:, h, ts_, :], smax[h][:, :, :].to_broadcast((P, g, H))
                )
                engine.dma_start(out=yv[h][:, ts_, :], in_=ot)

            # Stage a: stream in + reduce columns 0:128.
            ka = [0, sum(-(-g // 8) for _, g in IN_GROUPS_A)]
            for gi, (t0, g) in enumerate(IN_GROUPS_A):
                emit_mms(0, in_group(0, gi, t0, g, ACT_GROUPS_A), ka)
            scale_chain(0)

            # Middle: stage-a output (SWDGE queue) overlaps stage-b input (SP).
            kb = [0, sum(-(-g // 8) for _, g in IN_GROUPS_B)]
            for j in range(len(IN_GROUPS_B)):
                if j < len(OUT_GROUPS):
                    out_group(0, *OUT_GROUPS[j], engine=nc.sync)
                gi, (t0, g) = j, IN_GROUPS_B[j]
                emit_mms(1, in_group(1, gi, t0, g, ACT_GROUPS_B), kb)
            for j in range(len(IN_GROUPS_B), len(OUT_GROUPS)):
                out_group(0, *OUT_GROUPS[j], engine=nc.sync)
            scale_chain(1)

            # Tail: stage-b output on SP (its in-DMAs are done). One g=16
            # group's mul runs on the Pool engine to probe its rate.
            for t0, g in OUT_GROUPS:
                out_group(1, t0, g, engine=nc.sync)
    nc.compile()
    return nc


def _get_nc() -> bass.Bass:
    global _NC
    if _NC is None:
        _NC = _build()
    return _NC


def _shard_inputs(x: np.ndarray) -> list[dict]:
    xh = x.astype(np.float16)
    out = []
    for i in range(NCORES):
        sh = xh[:, i * C : (i + 1) * C]
        out.append({"x": np.ascontiguousarray(np.vstack([sh[:, :H], sh[:, H:]]))})
    return out


def kernel(x) -> np.ndarray:
    x = np.asarray(x, dtype=np.float32)
    assert x.shape == (N, D), x.shape
    nc = _get_nc()
    in_maps = _shard_inputs(x)
    try:
        res = run_bass_kernel_spmd(nc, in_maps, core_ids=list(range(NCORES)))
    except Exception:
        # Transient NRT/device hiccups (e.g. a previous process's profiling
        # session left a core wedged) recover after a short pause.
        import time

        time.sleep(5)
        res = run_bass_kernel_spmd(nc, in_maps, core_ids=list(range(NCORES)))
    outs = []
    for r in res.results:
        yh = r["y"]
        outs.append(np.hstack([yh[:N], yh[N:]]).astype(np.float32))
    return np.concatenate(outs, axis=1)


# revision 29
# speedup vs baseline: 1.0676x; 1.0676x over previous
"""Column-wise RMS normalization on 8 Trainium2 NeuronCores.

Computes y = x * rsqrt(sum(x*x, axis=0) + eps) for x [32768, 2048] f32.

Sharding: column-parallel — each core owns 256 columns (no collectives).
The host casts to fp16 (rel-err budget 2e-2 dwarfs fp16 rounding) and
re-packs the core's shard as two stacked column-half planes
[2N, 128] = [rows of cols 0:128; rows of cols 128:256], so every DMA
group [128p, g, 128c] moves 4KB contiguous runs per partition. HBM
traffic is 16MB in + 16MB out per core, the bandwidth floor.

Two-stage column-half pipeline: stage a (cols 0:128) streams in and
reduces; while stage b streams in, stage a's scaled output streams OUT
simultaneously on a different HWDGE queue — input and output DMA
overlap, hiding the square/reduce drain and the scale latency that a
single-stage schedule exposes as a dead-DMA gap.

Per stage:
  pass A: squares into fp8(e4m3) scratch (ACT ~141G elem/s takes most
          groups, DVE ~118G the ramp/tail); column reduction on TensorE
          via fp8 DoubleRow ones-matmuls (8 rows per matmul, Num=2 AP),
          with M=128 all-ones weights so u lands on EVERY partition of
          PSUM [128, 512] — the scale chain needs no broadcast hop.
  scale:  fold 4 interleaved partials (DVE) + Sqrt(+eps) (ACT) +
          reciprocal into a [128, 1, 128] fp16 tile (DVE-local tail).
  pass B: y = x*s on DVE tensor_tensor (~230G elem/s) with the scale
          broadcast over the row axis from SBUF (full rate), fp16 out.

Queues: all in-DMAs + stage-b out-DMAs on the SP HWDGE queue; stage-a
out-DMAs on the ACT HWDGE queue, concurrent with stage-b's in-DMAs.
"""

import numpy as np

import concourse.bacc as bacc
import concourse.bass as bass
import concourse.tile as tile
from concourse import mybir
from concourse.bass_utils import run_bass_kernel_spmd

N, D = 32768, 2048
EPS = 1e-6
NCORES = 8
C = D // NCORES  # 256 columns per core
H = 128          # column-half width
P = 128          # partitions
T = N // P       # 256 rows per partition
G = 32           # row-group (t) per DMA / compute chunk

# Stage a ramps up small (fast square->matmul pipeline spin-up, PE is
# the scale gate) and ends small (short final chain); stage b starts
# warm and ends small. ACT takes ~half the big groups' squares.
IN_GROUPS_A = [
    (0, 4), (4, 8), (12, 16), (28, 32), (60, 32), (92, 32), (124, 32),
    (156, 32), (188, 32), (220, 16), (236, 8), (244, 8), (252, 2), (254, 2),
]
ACT_GROUPS_A = {2, 4, 6, 7, 8}
IN_GROUPS_B = [(j * G, G) for j in range(7)] + [
    (224, 16), (240, 8), (248, 4), (252, 2), (254, 2),
]
ACT_GROUPS_B = {0, 1, 2, 3, 4, 5, 6, 7, 8, 9, 10, 11}
OUT_GROUPS = [(0, 8), (8, 8), (16, 16)] + [(32 + j * G, G) for j in range(6)] + [(224, 16), (240, 8), (248, 4), (252, 4)]

_NC = None


def _build() -> bass.Bass:
    nc = bacc.Bacc("TRN2", target_bir_lowering=False, enable_partition_id=False)
    x = nc.dram_tensor("x", [2 * N, H], mybir.dt.float16, kind="ExternalInput")
    y = nc.dram_tensor("y", [2 * N, H], mybir.dt.float16, kind="ExternalOutput")
    xv = x[:, :].rearrange("(h p t) c -> h p t c", h=2, p=P)
    yv = y[:, :].rearrange("(h p t) c -> h p t c", h=2, p=P)

    with tile.TileContext(nc) as tc:
        with (
            tc.tile_pool(name="cache", bufs=1) as cachep,
            tc.tile_pool(name="consts", bufs=1) as consts,
            tc.tile_pool(name="sq", bufs=2) as sqp,
            tc.tile_pool(name="outs", bufs=4) as outp,
            tc.tile_pool(name="scale", bufs=1) as scalep,
            tc.tile_pool(name="ps", bufs=1, space="PSUM") as psp,
        ):
            xc = cachep.tile([P, 2, T, H], mybir.dt.float16)
            ones2 = consts.tile([P, 2, P], mybir.dt.float8e4)
            nc.vector.memset(ones2, 1.0)
            eps_t = consts.tile([P, 1], mybir.dt.float32)
            nc.vector.memset(eps_t, EPS)
            # Pre-warm ACT function tables off the critical path.
            warm = consts.tile([1, 1], mybir.dt.float32)
            nc.scalar.activation(
                out=warm, in_=eps_t[0:1, 0:1],
                func=mybir.ActivationFunctionType.Square,
            )
            nc.scalar.activation(
                out=warm, in_=eps_t[0:1, 0:1],
                func=mybir.ActivationFunctionType.Sqrt,
            )

            u_ps = [psp.tile([P, 4 * H], mybir.dt.float32, name=f"u_ps{h}") for h in range(2)]
            smax = [scalep.tile([P, 1, H], mybir.dt.float16, name=f"smax{h}") for h in range(2)]


            def in_group(h, gi, t0, g, act_groups):
                ts_ = slice(t0, t0 + g)
                nc.sync.dma_start(out=xc[:, h, ts_, :], in_=xv[h][:, ts_, :])
                if g > 2:
                    sq = sqp.tile([P, g, H], mybir.dt.float8e4, tag="sq", bufs=5)
                else:
                    sq = sqp.tile([P, g, H], mybir.dt.float8e4, tag="sqt", bufs=4)
                if gi in act_groups:
                    nc.scalar.activation(
                        out=sq,
                        in_=xc[:, h, ts_, :],
                        func=mybir.ActivationFunctionType.Square,
                    )
                else:
                    nc.vector.tensor_mul(sq, xc[:, h, ts_, :], xc[:, h, ts_, :])
                # 8-row DoubleRow matmuls: rhs viewed [P, 2, (r/2)*H] (the
                # required Num=2 AP) pairs row t with t+r/2; accumulates
                # r/2 interleaved 128-col partials into u_ps[:, :inner].
                mm = []
                for r0 in range(0, g, 8):
                    r = min(8, g - r0)
                    view = sq[:, r0 : r0 + r, :].rearrange(
                        "p (two t) c -> p two (t c)", two=2
                    )
                    mm.append((view, (r // 2) * H))
                return mm

            def scale_chain(h):
                # u is redundantly materialized on every partition (M=128
                # ones), so the scale is all-local: fold 4 partials, sqrt,
                # reciprocal -- no PE broadcast hop.
                u2 = scalep.tile([P, 1, H], mybir.dt.float32, name=f"u{h}")
                uview = u_ps[h][:, :].rearrange("p (q c) -> p c q", q=4)
                nc.vector.reduce_sum(u2[:, 0, :], uview, axis=mybir.AxisListType.X)
                tsq = scalep.tile([P, 1, H], mybir.dt.float32, name=f"t{h}")
                nc.scalar.activation(
                    out=tsq[:, 0, :],
                    in_=u2[:, 0, :],
                    func=mybir.ActivationFunctionType.Sqrt,
                    bias=eps_t[:, 0:1],
                    scale=1.0,
                )
                s32 = scalep.tile([P, 1, H], mybir.dt.float32, name=f"s32_{h}")
                nc.vector.reciprocal_approx_fast(out=s32[:, :, :], in_=tsq[:, :, :])
                nc.vector.tensor_copy(smax[h], s32[:, :, :])

            def emit_mms(h, mm, kstate):
                for rhs, inner in mm:
                    nc.tensor.matmul(
                        u_ps[h][:, :inner],
                        lhsT=ones2[:, :, :],
                        rhs=rhs,
                        start=(kstate[0] == 0),
                        stop=(kstate[0] == kstate[1] - 1),
                        perf_mode=mybir.MatmulPerfMode.DoubleRow,
                    )
                    kstate[0] += 1

            def out_group(h, t0, g, engine, mul_engine=None):
                ts_ = slice(t0, t0 + g)
                ot = outp.tile([P, g, H], mybir.dt.float16, tag="ot", bufs=6)
                (mul_engine or nc.vector).tensor_mul(
                    ot, xc[:, h, ts_, :], smax[h][:, :, :].to_broadcast((P, g, H))
                )
                engine.dma_start(out=yv[h][:, ts_, :], in_=ot)

            # Stage a: stream in + reduce columns 0:128.
            ka = [0, sum(-(-g // 8) for _, g in IN_GROUPS_A)]
            for gi, (t0, g) in enumerate(IN_GROUPS_A):
                emit_mms(0, in_group(0, gi, t0, g, ACT_GROUPS_A), ka)
            scale_chain(0)

            # Middle: stage-a output (SWDGE queue) overlaps stage-b input (SP).
            kb = [0, sum(-(-g // 8) for _, g in IN_GROUPS_B)]
            for j in range(len(IN_GROUPS_B)):
                if j < len(OUT_GROUPS):
                    out_group(0, *OUT_GROUPS[j], engine=nc.scalar)
                gi, (t0, g) = j, IN_GROUPS_B[j]
                emit_mms(1, in_group(1, gi, t0, g, ACT_GROUPS_B), kb)
            for j in range(len(IN_GROUPS_B), len(OUT_GROUPS)):
                out_group(0, *OUT_GROUPS[j], engine=nc.scalar)
            scale_chain(1)

            # Tail: stage-b output on SP (its in-DMAs are done). One g=16
            # group's mul runs on the Pool engine to probe its rate.
            for t0, g in OUT_GROUPS:
                out_group(1, t0, g, engine=nc.sync)
    nc.compile()
    return nc


def _get_nc() -> bass.Bass:
    global _NC
    if _NC is None:
        _NC = _build()
    return _NC


def _shard_inputs(x: np.ndarray) -> list[dict]:
    xh = x.astype(np.float16)
    out = []
    for i in range(NCORES):
        sh = xh[:, i * C : (i + 1) * C]
        out.append({"x": np.ascontiguousarray(np.vstack([sh[:, :H], sh[:, H:]]))})
    return out


def kernel(x) -> np.ndarray:
    x = np.asarray(x, dtype=np.float32)
    assert x.shape == (N, D), x.shape
    nc = _get_nc()
    in_maps = _shard_inputs(x)
    try:
        res = run_bass_kernel_spmd(nc, in_maps, core_ids=list(range(NCORES)))
    except Exception:
        # Transient NRT/device hiccups (e.g. a previous process's profiling
        # session left a core wedged) recover after a short pause.
        import time

        time.sleep(5)
        res = run_bass_kernel_spmd(nc, in_maps, core_ids=list(range(NCORES)))
    outs = []
    for r in res.results:
        yh = r["y"]
        outs.append(np.hstack([yh[:N], yh[N:]]).astype(np.float32))
    return np.concatenate(outs, axis=1)
